# revision 1
# baseline (speedup 1.0000x reference)
"""Trainium2 Bass kernel for nn_AttentionLayer (B=16, V=1024, D=512, H=8, MAXHOP=8).

Sharding: data-parallel over batch B across 8 NeuronCores (2 batches/core).
The relative-position-bias gather (rpe[:, hop_matrix] -> [H,V,V]) is sharded
by head: core c builds head c's bias table on-chip (DVE select-accumulate),
then an AllGather distributes all 8 heads to every core. All other compute is
local to each core (zero collectives on the attention path).

Per-core math (transposed-score layout, so softmax lands on native axes):
  qT/kT = (W @ x^T) per head  [HD, tokens]   (bf16, fp32 accum)
  S_T[j,i] = k_j . q_i * scale + bias_T[j,i] (bias added into PSUM via an
             identity-matmul; bias_T streamed from the AllGather result)
  P_T = exp(S_T)                              (ScalarE, PSUM -> SBUF bf16)
  att_T[d,i] (+ denom row) = [v | 1]^T @ P_T  (ones-augmented V gives softmax
                                               denominators as an extra row)
  att = att_T * (1/denom)                     (reciprocal + DMA row-broadcast)
  out = att @ Wo^T + bo                       (per-head K=64 accumulation,
                                               bo via a K=1 ones matmul)
"""

import numpy as np

import concourse.bass as bass
import concourse.tile as tile
from concourse import bacc, mybir
from concourse.masks import make_identity

FP32 = mybir.dt.float32
BF16 = mybir.dt.bfloat16
INT32 = mybir.dt.int32

N_CORES = 8
B, V, D, H, NHOP = 16, 1024, 512, 8, 9


class Cfg:
    def __init__(self, NC, B, V, D, H, NHOP):
        self.NC, self.B, self.V, self.D, self.H, self.NHOP = NC, B, V, D, H, NHOP
        assert B % NC == 0 and H == NC
        self.BPC = B // NC           # batches per core
        self.HD = D // H             # head dim
        self.T = self.BPC * V        # tokens per core
        assert D % 128 == 0 and V % 128 == 0
        self.DCH = D // 128          # contraction chunks for projections
        self.NJT = V // 128          # key-position tiles
        self.SCH = min(512, V)       # S free-dim chunk
        self.NSC = V // self.SCH
        self.ICH = min(512, V)       # PV free-dim chunk
        self.NIC = V // self.ICH
        self.TCH = min(512, self.T)  # projection token chunk
        self.NTC = self.T // self.TCH
        self.NTT = self.T // 128     # token tiles
        assert self.HD <= 64 and 128 % self.HD == 0
        self.HPC = 128 // self.HD    # heads per 128-row chunk (=2)


def build_graph(tc, outs, ins, cfg):
    """Emit the per-core graph. `ins` is a dict name->AP of DRAM inputs,
    `outs` a single DRAM AP [BPC, V, D] f32."""
    from contextlib import ExitStack

    ctx = ExitStack()
    nc = tc.nc
    c = cfg
    xT_d, WqT_d, WkT_d, WvT_d = ins["xT"], ins["WqT"], ins["WkT"], ins["WvT"]
    WoTh_d, bo_d, rpeh_d, hopT_d = ins["WoTh"], ins["bo"], ins["rpeh"], ins["hopT"]
    out_d = outs

    scale = 1.0 / float(np.sqrt(c.HD))

    consts = ctx.enter_context(tc.tile_pool(name="consts", bufs=1))
    persist = ctx.enter_context(tc.tile_pool(name="persist", bufs=1))
    dram = ctx.enter_context(tc.tile_pool(name="dram", bufs=1, space="DRAM"))
    dram2 = ctx.enter_context(tc.tile_pool(name="dram2", bufs=3, space="DRAM"))

    # ---- constants -------------------------------------------------------
    ident = consts.tile([128, 128], BF16, name="ident")
    make_identity(nc, ident[:])
    ones_col = consts.tile([1, 128], BF16, name="ones_col")
    nc.vector.memset(ones_col[:], 1.0)
    rpe_cols = consts.tile([128, c.NHOP], FP32, name="rpe_cols")
    nc.sync.dma_start(rpe_cols[:], rpeh_d.broadcast_to([128, c.NHOP]))
    bo_f = consts.tile([1, c.D], FP32, name="bo_f")
    nc.sync.dma_start(bo_f[:], bo_d)
    bo_bf = consts.tile([1, c.D], BF16, name="bo_bf")
    nc.vector.tensor_copy(bo_bf[:], bo_f[:])

    # ---- scoped: input staging + projection-only tensors ------------------
    ctx_w = ExitStack()
    stage = ctx_w.enter_context(tc.tile_pool(name="stage", bufs=3))
    wpool = ctx_w.enter_context(tc.tile_pool(name="wpool", bufs=1))

    CH = min(512, c.T)
    xT = [wpool.tile([128, c.T], BF16, name=f"xT{k}") for k in range(c.DCH)]
    for k in range(c.DCH):
        for t in range(c.T // CH):
            xf = stage.tile([128, CH], FP32, name="xf", tag="xf")
            nc.sync.dma_start(xf[:], xT_d[k * 128:(k + 1) * 128,
                                          t * CH:(t + 1) * CH])
            nc.vector.tensor_copy(xT[k][:, t * CH:(t + 1) * CH], xf[:])

    def load_w(d_ap, nm):
        w = [wpool.tile([128, c.D], BF16, name=f"{nm}{k}") for k in range(c.DCH)]
        for k in range(c.DCH):
            wf = stage.tile([128, c.D], FP32, name="wf", tag="wf")
            nc.sync.dma_start(wf[:], d_ap[k * 128:(k + 1) * 128, :])
            nc.vector.tensor_copy(w[k][:], wf[:])
        return w

    WqT = load_w(WqT_d, "WqT")
    WkT = load_w(WkT_d, "WkT")
    WvT = load_w(WvT_d, "WvT")
    # WoTh: [HD, H, D] per-head layout, base partition 0 (persists to the end)
    WoTh = persist.tile([c.HD, c.H, c.D], BF16, name="WoTh")
    for h in range(c.H):
        wof = stage.tile([c.HD, c.D], FP32, name="wof", tag="wof")
        nc.sync.dma_start(wof[:], WoTh_d[:, h, :])
        nc.vector.tensor_copy(WoTh[:, h, :], wof[:])

    # ---- bias build (own head) + AllGather -------------------------------
    ctx_bias = ExitStack()
    bias_pools = ctx_bias.enter_context(tc.tile_pool(name="biasb", bufs=2))
    half = (c.NJT // 2) * 128 if c.NJT > 1 else c.V
    nhalf = 2 if c.NJT > 1 else 1
    bias_local_h = [dram.tile([half, c.V], BF16, name=f"bias_local{i}")
                    for i in range(nhalf)]
    bias_all_h = [dram.tile([c.H, half, c.V], BF16, name=f"bias_all{i}",
                            addr_space="Shared" if c.NC > 4 else "Local")
                  for i in range(nhalf)]
    jt_half = (c.NJT // 2) if c.NJT > 1 else c.NJT
    for jt in range(c.NJT):
        hop_i = bias_pools.tile([128, c.V], INT32, name="hop_i", tag="hop_i")
        nc.sync.dma_start(hop_i[:], hopT_d[jt * 128:(jt + 1) * 128, :])
        hop_b = bias_pools.tile([128, c.V], BF16, name="hop_b", tag="hop_b")
        nc.vector.tensor_copy(hop_b[:], hop_i[:])
        acc = bias_pools.tile([128, c.V], BF16, name="bacc", tag="bacc")
        nc.vector.tensor_scalar(
            acc[:], hop_b[:], 0.0, rpe_cols[:, 0:1],
            mybir.AluOpType.is_equal, mybir.AluOpType.mult,
        )
        for m in range(1, c.NHOP):
            term = bias_pools.tile([128, c.V], BF16, name="bterm", tag="bterm")
            nc.vector.tensor_scalar(
                term[:], hop_b[:], float(m), rpe_cols[:, m:m + 1],
                mybir.AluOpType.is_equal, mybir.AluOpType.mult,
            )
            nc.vector.tensor_tensor(acc[:], acc[:], term[:], mybir.AluOpType.add)
        hi, jr = divmod(jt, jt_half)
        nc.sync.dma_start(
            bias_local_h[hi][jr * 128:(jr + 1) * 128, :], acc[:])
        if jr == jt_half - 1:
            nc.gpsimd.collective_compute(
                "AllGather",
                mybir.AluOpType.bypass,
                replica_groups=[list(range(c.NC))],
                ins=[bias_local_h[hi].opt()],
                outs=[bias_all_h[hi].opt()],
            )
    ctx_bias.close()

    # ---- projections ------------------------------------------------------
    ctx_proj = ExitStack()
    ps_proj = ctx_proj.enter_context(
        tc.tile_pool(name="ps_proj", bufs=2, space="PSUM"))

    def transposed_proj(W, nm):
        dst = [persist.tile([128, c.T], BF16, name=f"{nm}{q}")
               for q in range(c.DCH)]
        for q in range(c.DCH):
            for t in range(c.NTC):
                ps = ps_proj.tile([128, c.TCH], FP32, name="ps_p", tag="ps_p")
                for k in range(c.DCH):
                    nc.tensor.matmul(
                        ps[:], W[k][:, q * 128:(q + 1) * 128],
                        xT[k][:, t * c.TCH:(t + 1) * c.TCH],
                        start=(k == 0), stop=(k == c.DCH - 1),
                    )
                if nm == "qT":  # fold in 1/sqrt(HD)
                    nc.vector.tensor_scalar_mul(
                        dst[q][:, t * c.TCH:(t + 1) * c.TCH], ps[:], scale)
                else:
                    nc.vector.tensor_copy(
                        dst[q][:, t * c.TCH:(t + 1) * c.TCH], ps[:])
        return dst

    qT = transposed_proj(WqT, "qT")
    kT = transposed_proj(WkT, "kT")

    # v in normal layout, ones-augmented: vt[tt] = [128, H, HD+1]
    vt = [persist.tile([128, c.H, c.HD + 1], BF16, name=f"vt{tt}")
          for tt in range(c.NTT)]
    for tt in range(c.NTT):
        ps = ps_proj.tile([128, c.D], FP32, name="ps_v", tag="ps_v")
        for k in range(c.DCH):
            nc.tensor.matmul(
                ps[:], xT[k][:, tt * 128:(tt + 1) * 128], WvT[k][:],
                start=(k == 0), stop=(k == c.DCH - 1),
            )
        nc.vector.tensor_copy(
            vt[tt][:, :, 0:c.HD],
            ps[:].rearrange("p (h d) -> p h d", h=c.H),
        )
        nc.vector.memset(vt[tt][:, :, c.HD:c.HD + 1], 1.0)

    ctx_proj.close()
    ctx_w.close()

    # ---- attention core ---------------------------------------------------
    att_pool = ctx.enter_context(tc.tile_pool(name="attn", bufs=c.H * c.BPC))
    ctx_att = ExitStack()
    biast_pool = ctx_att.enter_context(
        tc.tile_pool(name="biast", bufs=c.NJT + 1))
    p_pool = ctx_att.enter_context(tc.tile_pool(name="psb", bufs=3))
    rec_pool = ctx_att.enter_context(tc.tile_pool(name="rec", bufs=2))
    ps_s_pool = ctx_att.enter_context(
        tc.tile_pool(name="ps_s", bufs=1, space="PSUM"))
    ps_att_pool = ctx_att.enter_context(
        tc.tile_pool(name="ps_att", bufs=4, space="PSUM"))

    assert c.HPC == 2
    att_n = {}
    for g in range(c.H // 2):
        hA, hB = 2 * g, 2 * g + 1
        biastA, biastB = [], []
        for jt in range(c.NJT):
            hi, jr = divmod(jt, jt_half)
            btA = biast_pool.tile([128, c.V], BF16, name="btA", tag="btA")
            nc.sync.dma_start(
                btA[:], bias_all_h[hi][hA, jr * 128:(jr + 1) * 128, :])
            biastA.append(btA)
            btB = biast_pool.tile([128, c.V], BF16, name="btB", tag="btB")
            nc.sync.dma_start(
                btB[:], bias_all_h[hi][hB, jr * 128:(jr + 1) * 128, :])
            biastB.append(btB)
        for b in range(c.BPC):
            t0 = b * c.V
            ps_attA = [ps_att_pool.tile([c.HD + 1, c.ICH], FP32, name="ps_aA",
                                        tag="ps_a") for _ in range(c.NIC)]
            ps_attB = [ps_att_pool.tile([c.HD + 1, c.ICH], FP32, name="ps_aB",
                                        tag="ps_a") for _ in range(c.NIC)]
            for jt in range(c.NJT):
                ps_sA = ps_s_pool.tile([128, c.V], FP32, name="ps_sA",
                                       tag="ps_sA")
                ps_sB = ps_s_pool.tile([128, c.V], FP32, name="ps_sB",
                                       tag="ps_sB")
                for sc in range(c.NSC):
                    sl = slice(sc * c.SCH, (sc + 1) * c.SCH)
                    jsl = slice(t0 + jt * 128, t0 + (jt + 1) * 128)
                    ssl = slice(t0 + sc * c.SCH, t0 + (sc + 1) * c.SCH)
                    # heads 2g / 2g+1 occupy disjoint row-halves of the PE
                    # array (kT/qT slices at base partition 0 vs 64) -> the
                    # two S matmuls run concurrently as row-tiles.
                    nc.tensor.matmul(
                        ps_sA[:, sl], kT[g][0:c.HD, jsl], qT[g][0:c.HD, ssl],
                        start=True, stop=False)
                    nc.tensor.matmul(
                        ps_sB[:, sl], kT[g][c.HD:128, jsl],
                        qT[g][c.HD:128, ssl], start=True, stop=False)
                    nc.tensor.matmul(
                        ps_sA[:, sl], ident[:], biastA[jt][:, sl],
                        start=False, stop=True)
                    nc.tensor.matmul(
                        ps_sB[:, sl], ident[:], biastB[jt][:, sl],
                        start=False, stop=True)
                p_sbA = p_pool.tile([128, c.V], BF16, name="p_sbA", tag="p_sb")
                nc.scalar.activation(
                    p_sbA[:], ps_sA[:], mybir.ActivationFunctionType.Exp)
                p_sbB = p_pool.tile([128, c.V], BF16, name="p_sbB", tag="p_sb")
                nc.scalar.activation(
                    p_sbB[:], ps_sB[:], mybir.ActivationFunctionType.Exp)
                for i in range(c.NIC):
                    nc.tensor.matmul(
                        ps_attA[i], vt[b * c.NJT + jt][:, hA, :],
                        p_sbA[:, i * c.ICH:(i + 1) * c.ICH],
                        start=(jt == 0), stop=(jt == c.NJT - 1))
                    nc.tensor.matmul(
                        ps_attB[i], vt[b * c.NJT + jt][:, hB, :],
                        p_sbB[:, i * c.ICH:(i + 1) * c.ICH],
                        start=(jt == 0), stop=(jt == c.NJT - 1))
            for hh, ps_att in ((hA, ps_attA), (hB, ps_attB)):
                den_sb = rec_pool.tile([c.HD + 1, c.V], FP32, name="den_sb",
                                       tag="den_sb")
                for i in range(c.NIC):
                    nc.vector.tensor_copy(
                        den_sb[c.HD:c.HD + 1, i * c.ICH:(i + 1) * c.ICH],
                        ps_att[i][c.HD:c.HD + 1, :])
                den_dram = dram2.tile([1, c.V], FP32, name="den_dram",
                                      tag="den_dram")
                nc.sync.dma_start(den_dram[:], den_sb[c.HD:c.HD + 1, :])
                rec_bc = rec_pool.tile([c.HD, c.V], FP32, name="rec_bc",
                                       tag="rec_bc")
                nc.sync.dma_start(rec_bc[:],
                                  den_dram[:].broadcast_to([c.HD, c.V]))
                nc.vector.reciprocal_approx_fast(rec_bc[:], rec_bc[:])
                at = att_pool.tile([c.HD, c.V], BF16, name="at", tag="at")
                for i in range(c.NIC):
                    sl = slice(i * c.ICH, (i + 1) * c.ICH)
                    nc.vector.tensor_tensor(
                        at[:, sl], ps_att[i][0:c.HD, :], rec_bc[:, sl],
                        mybir.AluOpType.mult)
                att_n[(hh, b)] = at

    ctx_att.close()

    # ---- output projection ------------------------------------------------
    ctx_out = ExitStack()
    outsb_pool = ctx_out.enter_context(tc.tile_pool(name="outsb", bufs=3))
    ps_o_pool = ctx_out.enter_context(
        tc.tile_pool(name="ps_o", bufs=2, space="PSUM"))
    for b in range(c.BPC):
        for tt in range(c.NJT):
            ps_o = ps_o_pool.tile([128, c.D], FP32, name="ps_o", tag="ps_o")
            for h in range(c.H):
                nc.tensor.matmul(
                    ps_o[:],
                    att_n[(h, b)][:, tt * 128:(tt + 1) * 128],
                    WoTh[:, h, :],
                    start=(h == 0), stop=False,
                )
            nc.tensor.matmul(ps_o[:], ones_col[:], bo_bf[:],
                             start=False, stop=True)
            o_sb = outsb_pool.tile([128, c.D], FP32, name="o_sb", tag="o_sb")
            nc.vector.tensor_copy(o_sb[:], ps_o[:])
            nc.sync.dma_start(out_d[b, tt * 128:(tt + 1) * 128, :], o_sb[:])

    ctx_out.close()
    ctx.close()


# --------------------------------------------------------------------------
# Host side
# --------------------------------------------------------------------------

def shard_inputs(x, Wq, Wk, Wv, Wo, bo, rpe, hop_matrix, cfg):
    c = cfg
    WqT = np.ascontiguousarray(Wq.T.astype(np.float32))
    WkT = np.ascontiguousarray(Wk.T.astype(np.float32))
    WvT = np.ascontiguousarray(Wv.T.astype(np.float32))
    # Wo.T is [dv, do]; regroup per head at base partition 0: [HD, H, D]
    WoTh = np.ascontiguousarray(
        Wo.T.astype(np.float32).reshape(c.H, c.HD, c.D).transpose(1, 0, 2))
    hopT = np.ascontiguousarray(hop_matrix.T.astype(np.int32))
    bo2 = np.ascontiguousarray(bo.astype(np.float32).reshape(1, c.D))
    in_maps = []
    for core in range(c.NC):
        xs = x[core * c.BPC:(core + 1) * c.BPC].astype(np.float32)
        xT = np.ascontiguousarray(xs.reshape(c.T, c.D).T)
        in_maps.append({
            "xT": xT, "WqT": WqT, "WkT": WkT, "WvT": WvT, "WoTh": WoTh,
            "bo": bo2, "rpeh": np.ascontiguousarray(
                rpe[core:core + 1].astype(np.float32)),
            "hopT": hopT,
        })
    return in_maps


_CACHE = {}


def _get_compiled(cfg):
    key = (cfg.NC, cfg.B, cfg.V, cfg.D, cfg.H, cfg.NHOP)
    if key in _CACHE:
        return _CACHE[key]
    c = cfg
    nc = bacc.Bacc("TRN2", target_bir_lowering=False, debug=False,
                   num_devices=c.NC)
    ins = {
        "xT": nc.dram_tensor("xT", [c.D, c.T], FP32, kind="ExternalInput").ap(),
        "WqT": nc.dram_tensor("WqT", [c.D, c.D], FP32, kind="ExternalInput").ap(),
        "WkT": nc.dram_tensor("WkT", [c.D, c.D], FP32, kind="ExternalInput").ap(),
        "WvT": nc.dram_tensor("WvT", [c.D, c.D], FP32, kind="ExternalInput").ap(),
        "WoTh": nc.dram_tensor("WoTh", [c.HD, c.H, c.D], FP32,
                               kind="ExternalInput").ap(),
        "bo": nc.dram_tensor("bo", [1, c.D], FP32, kind="ExternalInput").ap(),
        "rpeh": nc.dram_tensor("rpeh", [1, c.NHOP], FP32,
                               kind="ExternalInput").ap(),
        "hopT": nc.dram_tensor("hopT", [c.V, c.V], INT32,
                               kind="ExternalInput").ap(),
    }
    out = nc.dram_tensor("out", [c.BPC, c.V, c.D], FP32,
                         kind="ExternalOutput").ap()
    with tile.TileContext(nc) as tc:
        build_graph(tc, out, ins, cfg)
    nc.compile()
    _CACHE[key] = nc
    return nc


def kernel(x, Wq, Wk, Wv, Wo, bo, rpe, hop_matrix):
    from concourse.bass_utils import run_bass_kernel_spmd

    cfg = Cfg(N_CORES, B, V, D, H, NHOP)
    nc = _get_compiled(cfg)
    in_maps = shard_inputs(np.asarray(x), np.asarray(Wq), np.asarray(Wk),
                           np.asarray(Wv), np.asarray(Wo), np.asarray(bo),
                           np.asarray(rpe), np.asarray(hop_matrix), cfg)
    res = run_bass_kernel_spmd(nc, in_maps, core_ids=list(range(cfg.NC)))
    return np.concatenate([res.results[c]["out"] for c in range(cfg.NC)],
                          axis=0)



# revision 15
# speedup vs baseline: 1.7761x; 1.7761x over previous
"""Trainium2 Bass kernel for nn_AttentionLayer (B=16, V=1024, D=512, H=8, MAXHOP=8).

Sharding: 4 head-groups x 2 batch-groups. Core c = 2*hg + bg handles heads
{2hg, 2hg+1} for batches bg*8..bg*8+7. The relative-position table is built
factored: w = exp(rpe)[hop] so that P = exp(S) * w (no bias-inject matmuls).
Core (hg, bg) builds head (2hg+bg)'s w-table on DVE and a 2-replica AllGather
{2hg, 2hg+1} exchanges the pair's tables (2 MB each way instead of the 16 MB
8-way gather of the data-parallel layout). Output partials (2 heads summed,
via one K=128 out-proj matmul over both heads) are summed across the 4
head-groups on the host, where bo is also added.

Per-core math (transposed-score layout):
  qT/kT = (W @ x^T) per head-pair [128, tokens] bf16 (q pre-scaled 1/sqrt(hd))
  S_h[j,i] = k_j . q_i            (row-paired K=64 matmuls, heads at PE rows
                                   0-63 / 64-127 run concurrently)
  P_h = exp(S_h)                  (ScalarE, PSUM -> SBUF bf16)
  P'_h = P_h * w_h[jt]            (DVE bf16 2x)
  att_T[d,i] (+denom row) = [v|1]^T @ P'_h   (ones-augmented V, M=65)
  att = att_T * (1/denom)         (approx-recip on the [1,V] row + DMA
                                   row-broadcast + DVE mult)
  out_part = [att_A; att_B] @ [WoT_A; WoT_B]  (single K=128 matmul per tile)
"""

import numpy as np

import concourse.bass as bass
import concourse.tile as tile
from concourse import bacc, mybir

FP32 = mybir.dt.float32
BF16 = mybir.dt.bfloat16

N_CORES = 8
B, V, D, H, NHOP = 16, 1024, 512, 8, 9
HG, BG = 4, 2                 # head groups x batch groups
HPC = H // HG                 # heads per core (=2)
BPC = B // BG                 # batches per core (=8)
HD = D // H                   # head dim (=64)
DH = HPC * HD                 # head-pair dims (=128)
T = BPC * V                   # tokens per core (=8192)
NJT = V // 128                # key-position tiles (=8)
NTT = T // 128                # token tiles (=64)
NTC = T // 512                # proj token chunks (=16)
DCH = D // 128                # contraction chunks (=4)


def build_graph(tc, out_d, ins, core_groups, dbg=None):
    from contextlib import ExitStack

    ctx = ExitStack()
    nc = tc.nc
    scale = 1.0 / float(np.sqrt(HD))

    xT_d, wq_d, wk_d, wv_d = ins["xT"], ins["WqTc"], ins["WkTc"], ins["WvTc"]
    woth_d, hop_d, rpeb_d = ins["WoTh"], ins["hopT"], ins["rpeb"]

    consts = ctx.enter_context(tc.tile_pool(name="consts", bufs=1))
    persist = ctx.enter_context(tc.tile_pool(name="persist", bufs=1))
    dram = ctx.enter_context(tc.tile_pool(name="dram", bufs=1, space="DRAM"))
    dram_rec = ctx.enter_context(tc.tile_pool(name="dram_rec", bufs=4,
                                              space="DRAM"))

    # ---- exp(rpe) row -> broadcast [128, NHOP] ---------------------------
    rpe_sb = consts.tile([1, NHOP], FP32, name="rpe_sb")
    nc.sync.dma_start(rpe_sb[:], rpeb_d)
    w9 = consts.tile([1, NHOP], FP32, name="w9")
    nc.scalar.activation(w9[:], rpe_sb[:], mybir.ActivationFunctionType.Exp)
    w9_dram = dram.tile([1, NHOP], FP32, name="w9_dram")
    nc.sync.dma_start(w9_dram[:], w9[:])
    wv9 = consts.tile([128, NHOP], FP32, name="wv9")
    nc.sync.dma_start(wv9[:], w9_dram[:].broadcast_to([128, NHOP]))

    # ---- persistent tensors ----------------------------------------------
    qT = persist.tile([DH, T], BF16, name="qT")
    kT = persist.tile([DH, T], BF16, name="kT")
    vt = persist.tile([128, NTT, HPC, HD + 1], BF16, name="vt")
    nc.vector.memset(vt[:, :, :, HD:HD + 1], 1.0)
    w_A = persist.tile([128, NJT * V], BF16, name="w_A")
    w_B = persist.tile([128, NJT * V], BF16, name="w_B")
    att_all = persist.tile([DH, T], BF16, name="att_all")
    woth = persist.tile([DH, D], BF16, name="woth")
    wof = consts.tile([DH, D], FP32, name="wof")
    nc.sync.dma_start(wof[:], woth_d)
    nc.vector.tensor_copy(woth[:], wof[:])

    own_dram = dram.tile([V, V], BF16, name="own_dram")
    gathered = dram.tile([HPC, V, V], BF16, name="gathered")

    # ---- scoped loading + build ------------------------------------------
    ctx_load = ExitStack()
    lpool = ctx_load.enter_context(tc.tile_pool(name="lpool", bufs=1))
    bpool = ctx_load.enter_context(tc.tile_pool(name="bpool", bufs=2))

    hop_sb = lpool.tile([128, NJT * V], BF16, name="hop_sb")
    for jt in range(NJT):
        nc.sync.dma_start(hop_sb[:, jt * V:(jt + 1) * V],
                          hop_d[jt * 128:(jt + 1) * 128, :])

    # w-table build for this core's assigned head (DVE select-accumulate)
    for jt in range(NJT):
        hsl = hop_sb[:, jt * V:(jt + 1) * V]
        acc = bpool.tile([128, V], BF16, name="bacc", tag="bacc")
        nc.vector.tensor_scalar(
            acc[:], hsl, 0.0, wv9[:, 0:1],
            mybir.AluOpType.is_equal, mybir.AluOpType.mult)
        for m in range(1, NHOP):
            term = bpool.tile([128, V], BF16, name="bterm", tag="bterm")
            nc.vector.tensor_scalar(
                term[:], hsl, float(m), wv9[:, m:m + 1],
                mybir.AluOpType.is_equal, mybir.AluOpType.mult)
            nc.vector.tensor_tensor(acc[:], acc[:], term[:],
                                    mybir.AluOpType.add)
        nc.sync.dma_start(own_dram[jt * 128:(jt + 1) * 128, :], acc[:])

    nc.gpsimd.collective_compute(
        "AllGather",
        mybir.AluOpType.bypass,
        replica_groups=core_groups,
        ins=[own_dram[:].opt()],
        outs=[gathered[:].opt()],
    )
    for jt in range(NJT):
        nc.sync.dma_start(w_A[:, jt * V:(jt + 1) * V],
                          gathered[0, jt * 128:(jt + 1) * 128, :])
        nc.sync.dma_start(w_B[:, jt * V:(jt + 1) * V],
                          gathered[1, jt * 128:(jt + 1) * 128, :])

    # ---- x load + projections --------------------------------------------
    xT_sb = lpool.tile([128, DCH, T], BF16, name="xT_sb")
    for kc in range(DCH):
        nc.sync.dma_start(xT_sb[:, kc, :], xT_d[kc * 128:(kc + 1) * 128, :])
    wq_sb = lpool.tile([128, DCH, DH], BF16, name="wq_sb")
    wk_sb = lpool.tile([128, DCH, DH], BF16, name="wk_sb")
    wv_sb = lpool.tile([128, DCH, DH], BF16, name="wv_sb")
    for d_ap, sb in ((wq_d, wq_sb), (wk_d, wk_sb), (wv_d, wv_sb)):
        for kc in range(DCH):
            nc.sync.dma_start(sb[:, kc, :], d_ap[kc * 128:(kc + 1) * 128, :])

    ctx_proj = ExitStack()
    ps_proj = ctx_proj.enter_context(
        tc.tile_pool(name="ps_proj", bufs=2, space="PSUM"))

    for tcn in range(NTC):
        tsl = slice(tcn * 512, (tcn + 1) * 512)
        ps_q = ps_proj.tile([DH, 512], FP32, name="ps_q", tag="ps_q")
        for kc in range(DCH):
            nc.tensor.matmul(ps_q[:], wq_sb[:, kc, :], xT_sb[:, kc, tsl],
                             start=(kc == 0), stop=(kc == DCH - 1))
        nc.scalar.mul(qT[:, tsl], ps_q[:], scale)
        ps_k = ps_proj.tile([DH, 512], FP32, name="ps_k", tag="ps_k")
        for kc in range(DCH):
            nc.tensor.matmul(ps_k[:], wk_sb[:, kc, :], xT_sb[:, kc, tsl],
                             start=(kc == 0), stop=(kc == DCH - 1))
        nc.scalar.copy(kT[:, tsl], ps_k[:])

    # v in token-major layout, ones-augmented
    for tt in range(NTT):
        ttsl = slice(tt * 128, (tt + 1) * 128)
        ps_v = ps_proj.tile([128, DH], FP32, name="ps_v", tag="ps_v")
        for kc in range(DCH):
            nc.tensor.matmul(ps_v[:], xT_sb[:, kc, ttsl], wv_sb[:, kc, :],
                             start=(kc == 0), stop=(kc == DCH - 1))
        nc.scalar.copy(
            vt[:, tt, :, 0:HD],
            ps_v[:].rearrange("p (h d) -> p h d", h=HPC))

    ctx_proj.close()
    ctx_load.close()

    # ---- attention --------------------------------------------------------
    ctx_att = ExitStack()
    ps_s_pool = ctx_att.enter_context(
        tc.tile_pool(name="ps_s", bufs=2, space="PSUM"))
    ps_att_pool = ctx_att.enter_context(
        tc.tile_pool(name="ps_att", bufs=2, space="PSUM"))
    p_pool = ctx_att.enter_context(tc.tile_pool(name="pp", bufs=3))
    p2_pool = ctx_att.enter_context(tc.tile_pool(name="p2p", bufs=4))
    rec_pool = ctx_att.enter_context(tc.tile_pool(name="recp", bufs=2))

    raw_pool = ctx_att.enter_context(tc.tile_pool(name="rawp", bufs=3))

    for b in range(BPC):
        t0 = b * V
        ps_att = {}
        for h in range(HPC):
            ps_att[h] = ps_att_pool.tile([HD + 1, V], FP32, name=f"ps_att{h}",
                                         tag="ps_att")
        p2s = {}

        def emit_pv(jt):
            for h in range(HPC):
                p2 = p2s.pop((jt, h))
                for ic in range(2):
                    isl = slice(ic * 512, (ic + 1) * 512)
                    nc.tensor.matmul(ps_att[h][:, isl],
                                     vt[:, b * NJT + jt, h, :], p2[:, isl],
                                     start=(jt == 0), stop=(jt == NJT - 1))

        for jt in range(NJT):
            jsl = slice(t0 + jt * 128, t0 + (jt + 1) * 128)
            # S pair: interleave A/B chunks so the K=64 row tiles (PE rows
            # 0-63 / 64-127) run concurrently.
            ps_sA = ps_s_pool.tile([128, V], FP32, name="ps_sA", tag="ps_s")
            ps_sB = ps_s_pool.tile([128, V], FP32, name="ps_sB", tag="ps_s")
            for sc in range(2):
                csl_ = slice(sc * 512, (sc + 1) * 512)
                ssl = slice(t0 + sc * 512, t0 + (sc + 1) * 512)
                nc.tensor.matmul(ps_sA[:, csl_], kT[0:HD, jsl], qT[0:HD, ssl],
                                 start=True, stop=True)
                nc.tensor.matmul(ps_sB[:, csl_], kT[HD:DH, jsl],
                                 qT[HD:DH, ssl], start=True, stop=True)
            for h, ps_s, w_t in ((0, ps_sA, w_A), (1, ps_sB, w_B)):
                p_sb = p_pool.tile([128, V], BF16, name="p_sb", tag="p")
                nc.scalar.activation(p_sb[:], ps_s[:],
                                     mybir.ActivationFunctionType.Exp)
                p2 = p2_pool.tile([128, V], BF16, name="p2", tag="p2")
                nc.vector.tensor_tensor(p2[:], p_sb[:],
                                        w_t[:, jt * V:(jt + 1) * V],
                                        mybir.AluOpType.mult)
                p2s[(jt, h)] = p2
            # software-pipelined PV: consume the previous jt's P' so the PE
            # never waits on the exp->mult chain of the current jt.
            if jt > 0:
                emit_pv(jt - 1)
        emit_pv(NJT - 1)

        # Drain ps_att to SBUF fast (frees PSUM for the next b), then
        # normalize: recip on the [1,V] den row, DRAM-bounce row broadcast,
        # DVE mult into a contiguous tmp, DMA into att_all (DVE lanes cannot
        # shift partitions; DMA can).
        for h in range(HPC):
            raw = raw_pool.tile([HD + 1, V], FP32, name="raw", tag="raw")
            nc.vector.tensor_copy(raw[:], ps_att[h][:])
            # den row lives at partition 64; DVE cannot shift partitions, so
            # DMA it down to partition 0 before the reciprocal.
            den0 = rec_pool.tile([1, V], FP32, name="den0", tag="den0")
            nc.sync.dma_start(den0[:], raw[HD:HD + 1, :])
            rrow = rec_pool.tile([1, V], FP32, name="rrow", tag="rrow")
            nc.vector.reciprocal_approx_fast(rrow[:], den0[:])
            rd = dram_rec.tile([1, V], FP32, name="rd", tag="rd")
            nc.sync.dma_start(rd[:], rrow[:])
            rbc = rec_pool.tile([HD, V], FP32, name="rbc", tag="rbc")
            nc.sync.dma_start(rbc[:], rd[:].broadcast_to([HD, V]))
            attH = rec_pool.tile([HD, V], BF16, name="attH", tag="attH")
            nc.vector.tensor_tensor(attH[:], raw[0:HD, :], rbc[:],
                                    mybir.AluOpType.mult)
            nc.sync.dma_start(att_all[h * HD:(h + 1) * HD, t0:t0 + V],
                              attH[:])

    ctx_att.close()

    if dbg is not None:
        dpool = ctx.enter_context(tc.tile_pool(name="dbgp", bufs=2))
        for nm, t in (("qT", qT), ("kT", kT), ("w_A", w_A), ("w_B", w_B),
                      ("att", att_all)):
            nslices = t.shape[1] // 4096 if t.shape[1] >= 4096 else 1
            for sidx in range(nslices):
                sl = slice(sidx * 4096, (sidx + 1) * 4096)
                nc.sync.dma_start(dbg[nm][:, sl], t[:, sl])

    # ---- output projection ------------------------------------------------
    ctx_out = ExitStack()
    outsb = ctx_out.enter_context(tc.tile_pool(name="outsb", bufs=4))
    ps_o_pool = ctx_out.enter_context(
        tc.tile_pool(name="ps_o", bufs=6, space="PSUM"))
    for b in range(BPC):
        for it in range(NJT):
            ps_o = ps_o_pool.tile([128, D], FP32, name="ps_o", tag="ps_o")
            nc.tensor.matmul(ps_o[:],
                             att_all[:, b * V + it * 128:b * V + (it + 1) * 128],
                             woth[:], start=True, stop=True)
            o_sb = outsb.tile([128, D], BF16, name="o_sb", tag="o_sb")
            if it % 2 == 0:
                nc.scalar.copy(o_sb[:], ps_o[:])
            else:
                nc.vector.tensor_copy(o_sb[:], ps_o[:])
            nc.sync.dma_start(out_d[b, it * 128:(it + 1) * 128, :], o_sb[:])
    ctx_out.close()
    ctx.close()


# --------------------------------------------------------------------------
# Host side
# --------------------------------------------------------------------------

def _bf16(a):
    import ml_dtypes
    return np.ascontiguousarray(a.astype(ml_dtypes.bfloat16))


def shard_inputs(x, Wq, Wk, Wv, Wo, bo, rpe, hop_matrix):
    x = np.asarray(x, np.float32)
    WqT = np.asarray(Wq, np.float32).T
    WkT = np.asarray(Wk, np.float32).T
    WvT = np.asarray(Wv, np.float32).T
    WoT = np.asarray(Wo, np.float32).T
    hopT = np.asarray(hop_matrix).T.astype(np.float32)
    rpe = np.asarray(rpe, np.float32)
    in_maps = []
    for c in range(N_CORES):
        hg, bg = c // BG, c % BG
        csl = slice(hg * DH, (hg + 1) * DH)
        xs = x[bg * BPC:(bg + 1) * BPC].reshape(T, D).T
        head_built = HPC * hg + bg
        in_maps.append({
            "xT": _bf16(xs),
            "WqTc": _bf16(WqT[:, csl]),
            "WkTc": _bf16(WkT[:, csl]),
            "WvTc": _bf16(WvT[:, csl]),
            "WoTh": np.ascontiguousarray(WoT[csl, :]),
            "hopT": _bf16(hopT),
            "rpeb": np.ascontiguousarray(rpe[head_built:head_built + 1, :]),
        })
    return in_maps


def unshard_output(results, bo):
    import ml_dtypes
    bo = np.asarray(bo, np.float32)
    outs = []
    for bg in range(BG):
        acc = np.zeros((BPC, V, D), np.float32)
        for hg in range(HG):
            acc += results[hg * BG + bg]["out"].astype(np.float32)
        outs.append(acc + bo)
    return np.concatenate(outs, axis=0)


_CACHE = {}


def _get_compiled():
    if "nc" in _CACHE:
        return _CACHE["nc"]
    nc = bacc.Bacc("TRN2", target_bir_lowering=False, debug=False,
                   num_devices=N_CORES)
    ins = {
        "xT": nc.dram_tensor("xT", [D, T], BF16, kind="ExternalInput").ap(),
        "WqTc": nc.dram_tensor("WqTc", [D, DH], BF16,
                               kind="ExternalInput").ap(),
        "WkTc": nc.dram_tensor("WkTc", [D, DH], BF16,
                               kind="ExternalInput").ap(),
        "WvTc": nc.dram_tensor("WvTc", [D, DH], BF16,
                               kind="ExternalInput").ap(),
        "WoTh": nc.dram_tensor("WoTh", [DH, D], FP32,
                               kind="ExternalInput").ap(),
        "hopT": nc.dram_tensor("hopT", [V, V], BF16,
                               kind="ExternalInput").ap(),
        "rpeb": nc.dram_tensor("rpeb", [1, NHOP], FP32,
                               kind="ExternalInput").ap(),
    }
    out = nc.dram_tensor("out", [BPC, V, D], BF16,
                         kind="ExternalOutput").ap()
    core_groups = [[2 * g, 2 * g + 1] for g in range(HG)]
    import os
    dbg = None
    if os.environ.get("KBG_DEBUG"):
        dbg = {
            "qT": nc.dram_tensor("dbg_qT", [DH, T], BF16,
                                 kind="ExternalOutput").ap(),
            "kT": nc.dram_tensor("dbg_kT", [DH, T], BF16,
                                 kind="ExternalOutput").ap(),
            "w_A": nc.dram_tensor("dbg_w_A", [128, NJT * V], BF16,
                                  kind="ExternalOutput").ap(),
            "w_B": nc.dram_tensor("dbg_w_B", [128, NJT * V], BF16,
                                  kind="ExternalOutput").ap(),
            "att": nc.dram_tensor("dbg_att", [DH, T], BF16,
                                  kind="ExternalOutput").ap(),
        }
    with tile.TileContext(nc) as tc:
        build_graph(tc, out, ins, core_groups, dbg)
    nc.compile()
    _CACHE["nc"] = nc
    return nc


def kernel(x, Wq, Wk, Wv, Wo, bo, rpe, hop_matrix):
    from concourse.bass_utils import run_bass_kernel_spmd

    nc = _get_compiled()
    in_maps = shard_inputs(x, Wq, Wk, Wv, Wo, bo, rpe, hop_matrix)
    res = run_bass_kernel_spmd(nc, in_maps, core_ids=list(range(N_CORES)))
    return unshard_output(res.results, bo)
